# revision 31
# baseline (speedup 1.0000x reference)
"""Trainium2 Bass kernel for nn_BigramHash: out = tab[hash(t,prev)] @ w_proj.T.

Strategy (fold sharded by table rows, tokens routed to row owners, gather
done as a one-hot matmul from SBUF-resident tab2 — no HBM gather traffic):

  - The projection is folded into the table on-device with bf16 matmuls:
    tab2 = tab @ w_proj.T, sharded by table rows.  Rows are assigned to
    cores by a balanced partition (exactly 384 rows per core, token
    counts equalized to NTOK/8) so every core gets exactly `tiles`
    128-token tiles of output — no padding waste.
  - The host routes each token to the core owning its hashed row and
    ships the LOCAL row index per token as flat fp16 (the hash is pure
    routing/marshalling — the host already computes it for the argsort).
  - The fold and the gather both run in nq=2 column halves with w_proj
    loaded half-major: the first fold needs only half the weights, so
    the first output DMA starts after ~1.25 MB of input instead of
    2.25 MB.  Critical loads (tabT chunk 0 + w half 0) go on the SP
    HWDGE queue (dodging the ACT LoadActFuncSet); everything else loads
    on ACT, overlapping the output stream.
  - Per 128-token tile and half: a K=1 fp16 matmul broadcasts the
    tile's indices across partitions (PSUM), is_equal against a
    per-partition iota builds a one-hot matrix (bf16, kept in SBUF for
    both halves), and one-hot @ tab2_chunk matmuls materialize the
    gathered rows in PSUM; DVE/ACT copy them to SBUF and the SP queue
    streams them to HBM in 512 KB chunks (2 tiles x 1 half).  The
    output stream is the roofline term (~16.8 MB/core); everything
    else hides behind it.

HBM traffic per core: ~2.8 MB of weights/indices in + ~16.8 MB out.
"""

import numpy as np

import concourse.bass as bass
import concourse.tile as tile
from concourse import bacc, mybir
from concourse.bass_utils import run_bass_kernel_spmd

N_CORES = 8
B, T = 4, 8192
SZ, D = 3072, 1024
NTOK = B * T                      # 32768
SLICE = SZ // N_CORES             # 384 table rows per core
RC_LOC = SLICE // 128             # 3 row chunks per core
KC = D // 128                     # 8 contraction chunks
HW = D // 2                       # 512-column half

C_T = 31337 % SZ                  # 617
C_P = 1000003 % SZ                # 1603

_CACHE = {}


def declare_io(nc, tiles, nq=2):
    f32 = mybir.dt.float32
    f16 = mybir.dt.float16
    bf16 = mybir.dt.bfloat16
    cap = tiles * 128
    qw = D // nq
    # flat local row index per token, fp16 (exact: values <= 383)
    idxf_ap = nc.dram_tensor("idxf", [1, cap], f16, kind="ExternalInput").ap()
    # per-partition iota p + 128c for the one-hot compare
    iota_ap = nc.dram_tensor("iota", [128, RC_LOC], f32, kind="ExternalInput").ap()
    # tabT re-laid by row-chunk: [rc][k', kc*128 + r]
    tabT_ap = nc.dram_tensor(
        "tabT", [RC_LOC, 128, KC * 128], bf16, kind="ExternalInput"
    ).ap()
    # w_proj.T column-slice-major: [k', q*KC*qw + kc*qw + d']
    wTh_ap = nc.dram_tensor("wTh", [128, nq * KC * qw], bf16,
                            kind="ExternalInput").ap()
    # output, tile/column-slice-major: host untangles to token order
    out_ap = nc.dram_tensor(
        "out_sh", [tiles, 128, D], f32, kind="ExternalOutput"
    ).ap()
    return idxf_ap, iota_ap, tabT_ap, wTh_ap, out_ap


def emit_body(nc, tc, io, tiles, bmax=None, gather_bufs=6, oc=2,
              lookahead=2, fold_spread=3, wh_pieces=1, dve_head=0,
              out_rings=1, copy_split=1, opool_bufs=2, bpool_bufs=2,
              fpool_bufs=2, nq=2, first_small=0, tt0_act=1,
              fw_from=18, fc=1, split_first=0):
    f32 = mybir.dt.float32
    f16 = mybir.dt.float16
    bf16 = mybir.dt.bfloat16
    idxf_ap, iota_ap, tabT_ap, wTh_ap, out_ap = io
    cap = tiles * 128
    qw = D // nq
    if bmax is None:
        ranges = [(0, RC_LOC - 1)] * tiles
    else:
        ranges = list(bmax)
    GR = 4                        # tiles per index-broadcast group
    n_groups = -(-tiles // GR)
    grange = [
        (min(ranges[j][0] for j in range(g * GR, min((g + 1) * GR, tiles))),
         max(ranges[j][1] for j in range(g * GR, min((g + 1) * GR, tiles))))
        for g in range(n_groups)
    ]
    n_oh = sum(hi - lo + 1 for lo, hi in grange)
    with (
        tc.tile_pool(name="weights", bufs=1) as wpool,
        tc.tile_pool(name="idx", bufs=1) as ipool,
        tc.tile_pool(name="oh", bufs=n_oh) as ohpool,
        tc.tile_pool(name="gather", bufs=gather_bufs) as gpool,
        tc.tile_pool(name="bcast_ps", bufs=bpool_bufs, space="PSUM") as bpool,
        tc.tile_pool(name="fold_ps", bufs=fpool_bufs, space="PSUM") as fpool,
        tc.tile_pool(name="gather_ps", bufs=opool_bufs, space="PSUM") as opool,
    ):
        # ---- loads. SP queue: the h0-critical weights, then the output
        # stream.  ACT queue: LoadActFuncSet (framework preamble), the
        # small index/iota tensors, tabT rc1/rc2, then the h1 weights —
        # those overlap the h0 output stream on a separate HWDGE ring.
        tabT_sb = [
            wpool.tile([128, KC * 128], bf16, tag=f"tabT{rc}", name=f"tT{rc}")
            for rc in range(RC_LOC)
        ]
        wTh_sb = wpool.tile([128, nq * KC * qw], bf16, tag="wTh", name="wTh")
        if not tt0_act:
            nc.sync.dma_start(tabT_sb[0][:], tabT_ap[0])
        hstep = KC * qw // wh_pieces
        for pc in range(wh_pieces):
            nc.sync.dma_start(wTh_sb[:, pc * hstep:(pc + 1) * hstep],
                              wTh_ap[:, pc * hstep:(pc + 1) * hstep])
        if tt0_act:
            nc.scalar.dma_start(tabT_sb[0][:], tabT_ap[0])
        idx_sb = ipool.tile([1, cap], f16, name="idxf")
        nc.scalar.dma_start(idx_sb[:], idxf_ap[:])
        iota_sb = ipool.tile([128, RC_LOC], f32, name="iota")
        nc.scalar.dma_start(iota_sb[:], iota_ap[:])
        nc.scalar.dma_start(tabT_sb[1][:], tabT_ap[1])
        nc.scalar.dma_start(tabT_sb[2][:], tabT_ap[2])

        # weight loads for column slices q >= 1 are deferred into the
        # gather pass so the early pair copies (also on ACT) are not stuck
        # behind them in the in-order ACT queue; they still land long
        # before their pass needs them.
        def _wh1_piece(qpc):
            qq, pc = qpc
            o = qq * KC * qw
            nc.scalar.dma_start(wTh_sb[:, o + pc * hstep:o + (pc + 1) * hstep],
                                wTh_ap[:, o + pc * hstep:o + (pc + 1) * hstep])
        wh1_pending = [(qq, pc) for qq in range(1, nq)
                       for pc in range(wh_pieces)]

        ones_sb = wpool.tile([1, 128], f16)
        nc.vector.memset(ones_sb[:], 1.0)

        # PE p-state warmup: early dummy matmuls start the ramp clock so
        # the fold runs at full clock when its inputs land
        for _ in range(3):
            psw = bpool.tile([128, GR * 128], f32, name="psb", tag="psb")
            nc.tensor.matmul(psw[:, 0:128], ones_sb[0:1, 0:128], ones_sb[:],
                             start=True, stop=True)

        # ---- fold: tab2[rc][:, h*512:...] = tab[rows rc] @ w_proj.T[h]
        # h-major queue: all of h0 (rc0, rc1, rc2) before h1, emitted
        # incrementally so the PE pipeline never pauses for a burst.
        tab2_sb = [
            wpool.tile([128, D], bf16, tag=f"tab2_{rc}", name=f"tab2_{rc}")
            for rc in range(RC_LOC)
        ]
        fold_queue = [(h, rc, kc)
                      for h in range(nq) for rc in range(RC_LOC)
                      for kc in range(KC)]
        fold_state = {"pos": 0, "ps": None, "ncopies": 0}

        def emit_fold(n=None, upto=None):
            while fold_state["pos"] < len(fold_queue):
                h, rc, kc = fold_queue[fold_state["pos"]]
                if upto is not None and (h, rc) > upto:
                    break
                if n is not None:
                    if n <= 0:
                        break
                    n -= 1
                if kc == 0:
                    fold_state["ps"] = fpool.tile([128, qw], f32, name="fps",
                                                  tag="fps")
                nc.tensor.matmul(
                    fold_state["ps"][:],
                    tabT_sb[rc][:, kc * 128:(kc + 1) * 128],
                    wTh_sb[:, h * KC * qw + kc * qw:h * KC * qw + (kc + 1) * qw],
                    start=(kc == 0), stop=(kc == KC - 1),
                )
                if kc == KC - 1:
                    # first copy (h0 rc0) on DVE — ACT is still loading;
                    # the rest on ACT once its queue drains
                    eng = nc.vector if fold_state["ncopies"] == 0 else nc.scalar
                    if eng is nc.scalar:
                        eng.copy(tab2_sb[rc][:, h * qw:(h + 1) * qw],
                                 fold_state["ps"][:])
                    else:
                        eng.tensor_copy(tab2_sb[rc][:, h * qw:(h + 1) * qw],
                                        fold_state["ps"][:])
                    fold_state["ncopies"] += 1
                fold_state["pos"] += 1

        # ---- one-hot selection matrices, kept in SBUF for both halves ----
        ohs = {}

        def emit_bcast_eq(g):
            gn = min(GR, tiles - g * GR)
            psb = bpool.tile([128, GR * 128], f32, name="psb", tag="psb")
            nc.tensor.matmul(
                psb[:, 0:gn * 128], ones_sb[0:1, 0:128],
                idx_sb[0:1, g * GR * 128:(g * GR + gn) * 128],
                start=True, stop=True,
            )
            glo, ghi = grange[g]
            ohs[g] = {}
            for c in range(glo, ghi + 1):
                oh = ohpool.tile([128, GR * 128], bf16, name="oh", tag="oh")
                nc.vector.tensor_tensor(
                    oh[:, 0:gn * 128], psb[:, 0:gn * 128],
                    iota_sb[:, c:c + 1].to_broadcast([128, gn * 128]),
                    op=mybir.AluOpType.is_equal,
                )
                ohs[g][c] = oh

        emit_fold(upto=(0, 0))

        # ---- gather schedule ----
        # Default: all tiles at h=0, then h=1 (half-width, two phases).
        # With fw_from=S (nq=2): tiles 0..S-1 run half-width in two
        # phases (early stream start while the h1 weights load), tiles
        # S.. run full-width in a single pass — full-tile output DMAs
        # write 4 KB per-partition runs (half the descriptor count).
        def chunk_sched(h, jlo, jhi, step):
            sched, j0 = [], jlo
            if h == 0:
                for _ in range(first_small):
                    if j0 < jhi:
                        sched.append((j0, 1))
                        j0 += 1
            while j0 < jhi:
                k = min(step, jhi - j0)
                sched.append((j0, k))
                j0 += k
            return sched

        copy_engs = [nc.vector, nc.scalar]
        lookahead = max(1, min(lookahead, n_groups))
        for g in range(lookahead):
            emit_bcast_eq(g)
        state = {"unit": 0, "chunk": 0, "item": 0}

        def ensure_oh(gi):
            need = min(gi + lookahead, n_groups - 1)
            while len(ohs) <= need:
                emit_bcast_eq(len(ohs))

        def copy_eng():
            if state["unit"] < dve_head:
                eng = nc.vector
            else:
                eng = copy_engs[state["unit"] % 2]
            state["unit"] += 1
            if wh1_pending and state["unit"] >= 2:
                _wh1_piece(wh1_pending.pop(0))
            return eng

        def emit_copy(eng, dst, src):
            if eng is nc.scalar:
                eng.copy(dst, src)
            else:
                eng.tensor_copy(dst, src)

        def out_ring():
            ring_engs = [nc.sync, nc.scalar][:out_rings]
            eng = ring_engs[state["chunk"] % len(ring_engs)]
            state["chunk"] += 1
            return eng

        def spread(early_gate):
            if state["item"] >= 1:
                gate = ((0, RC_LOC - 1)
                        if (early_gate and state["item"] < 12) else None)
                emit_fold(n=fold_spread, upto=gate)
            state["item"] += 1

        def half_phase(h, jlo, jhi):
            sched = chunk_sched(h, jlo, jhi, oc)
            tile2chunk = {}
            for ci, (s, k) in enumerate(sched):
                for j in range(s, s + k):
                    tile2chunk[j] = ci
            g_for_chunk = {}
            ps = None
            pk = 1
            for j in range(jlo, jhi):
                gi, off = j // GR, j % GR
                cmin, cmax = ranges[j]
                emit_fold(upto=(h, cmax))
                jj = (j - jlo) % 2
                if jj == 0:
                    pk = min(2, jhi - j)
                    ps = opool.tile([128, 2 * qw], f32, name="ps")
                for c in range(cmin, cmax + 1):
                    nc.tensor.matmul(
                        ps[:, jj * qw:(jj + 1) * qw],
                        ohs[gi][c][:, off * 128:(off + 1) * 128],
                        tab2_sb[c][:, h * qw:(h + 1) * qw],
                        start=(c == cmin), stop=(c == cmax),
                    )
                spread(h == 0)
                ensure_oh(gi)
                # copy unit = contiguous tiles within one PSUM pair AND
                # one DMA chunk; fires at whichever boundary comes first
                ci = tile2chunk[j]
                s, k = sched[ci]
                pair_start = j - jj
                pair_end = min(pair_start + pk - 1, jhi - 1)
                chunk_end = s + k - 1
                if j == min(pair_end, chunk_end):
                    unit_start = max(pair_start, s)
                    if ci not in g_for_chunk:
                        g_for_chunk[ci] = gpool.tile([128, oc * qw], f32,
                                                     name="g_sb")
                    g_sb = g_for_chunk[ci]
                    gb = (unit_start - s) * qw
                    pb = (unit_start - pair_start) * qw
                    width = j - unit_start + 1
                    if split_first and state["unit"] == 0 and width == 2:
                        # split the very first unit across both engines so
                        # the first out DMA waits a half-size copy
                        e1, e2 = copy_eng(), copy_eng()
                        emit_copy(e1, g_sb[:, gb:gb + qw],
                                  ps[:, pb:pb + qw])
                        emit_copy(e2, g_sb[:, gb + qw:gb + 2 * qw],
                                  ps[:, pb + qw:pb + 2 * qw])
                    else:
                        emit_copy(copy_eng(),
                                  g_sb[:, gb:(j - s + 1) * qw],
                                  ps[:, pb:(j - pair_start + 1) * qw])
                if j == chunk_end:
                    g_sb = g_for_chunk.pop(ci)
                    out_ring().dma_start(
                        out_ap[s:s + k, :, h * qw:(h + 1) * qw]
                        .rearrange("k p d -> p k d"),
                        g_sb[:, 0:k * qw].rearrange("p (k d) -> p k d",
                                                    k=k),
                    )

        def full_span(jlo):
            g_sb = None
            fk = 1
            for j in range(jlo, tiles):
                gi, off = j // GR, j % GR
                cmin, cmax = ranges[j]
                emit_fold(upto=(nq - 1, cmax))
                ps = opool.tile([128, 2 * qw], f32, name="ps")
                for hh in range(2):
                    for c in range(cmin, cmax + 1):
                        nc.tensor.matmul(
                            ps[:, hh * qw:(hh + 1) * qw],
                            ohs[gi][c][:, off * 128:(off + 1) * 128],
                            tab2_sb[c][:, hh * qw:(hh + 1) * qw],
                            start=(c == cmin), stop=(c == cmax),
                        )
                spread(False)
                ensure_oh(gi)
                qj = (j - jlo) % fc
                if qj == 0:
                    fk = min(fc, tiles - j)
                    g_sb = gpool.tile([128, fc * D], f32, name="g_sb")
                emit_copy(copy_eng(), g_sb[:, qj * D:(qj + 1) * D], ps[:])
                if qj == fk - 1:
                    j0 = j - qj
                    out_ring().dma_start(
                        out_ap[j0:j0 + fk].rearrange("k p d -> p k d"),
                        g_sb[:, 0:fk * D].rearrange("p (k d) -> p k d",
                                                    k=fk),
                    )

        if fw_from is not None and nq == 2 and 0 < fw_from < tiles:
            S = min(fw_from + (fw_from % 2), tiles)
            half_phase(0, 0, S)
            half_phase(1, 0, S)
            full_span(S)
        else:
            for h in range(nq):
                half_phase(h, 0, tiles)
        emit_fold()


def build(tiles, loop_iters=None, bmax=None, unroll=1, **body_kw):
    """Build the SPMD Bass program (same program for all 8 cores)."""
    key = ("nc", tiles, loop_iters, bmax, unroll, tuple(sorted(body_kw.items())))
    if key in _CACHE:
        return _CACHE[key]
    nc = bacc.Bacc("TRN2", target_bir_lowering=False, debug=False)
    io = declare_io(nc, tiles, nq=body_kw.get("nq", 2))
    with tile.TileContext(nc) as tc:
        if loop_iters is None:
            emit_body(nc, tc, io, tiles, bmax=bmax, **body_kw)
        else:
            with tc.For_i(0, loop_iters, 1):
                for _ in range(unroll):
                    emit_body(nc, tc, io, tiles, bmax=bmax, **body_kw)
    nc.compile()
    _CACHE[key] = nc
    return nc


def _hash_idx_host(t_flat, p_flat):
    a = (t_flat.astype(np.int64) % SZ) * C_T
    b = (p_flat.astype(np.int64) % SZ) * C_P
    return ((a + b) % SZ).astype(np.int64)


def _balance_rows(row_counts):
    """Partition SZ rows into N_CORES sets of exactly SLICE rows with token
    counts as equal as possible (ideally == NTOK/N_CORES each)."""
    target = int(row_counts.sum()) // N_CORES
    order = np.argsort(-row_counts, kind="stable")
    lists = [[] for _ in range(N_CORES)]
    sums = [0] * N_CORES
    for r in order:
        best = min(
            (c for c in range(N_CORES) if len(lists[c]) < SLICE),
            key=lambda c: (sums[c], c),
        )
        lists[best].append(int(r))
        sums[best] += int(row_counts[r])
    # pairwise swap repair toward max(sums) <= target
    for _ in range(400):
        hi = int(np.argmax(sums))
        if sums[hi] <= target:
            break
        lo = int(np.argmin(sums))
        want = min(sums[hi] - target, target - sums[lo])
        if want <= 0:
            break
        lo_vals = {}
        for bi, b in enumerate(lists[lo]):
            lo_vals.setdefault(int(row_counts[b]), bi)
        done = False
        for d in range(int(want), 0, -1):
            for ai, a in enumerate(lists[hi]):
                bi = lo_vals.get(int(row_counts[a]) - d)
                if bi is not None:
                    b = lists[lo][bi]
                    lists[hi][ai], lists[lo][bi] = b, a
                    sums[hi] -= d
                    sums[lo] += d
                    done = True
                    break
            if done:
                break
        if not done:
            break
    return [np.sort(np.array(l, dtype=np.int64)) for l in lists]


def route(t, tab=None, w_proj=None):
    """Host routing: balanced row->core map, tokens ordered by owning core
    then local row; returns order, per-core counts, capacity in tiles."""
    t = np.asarray(t)
    prev = np.pad(t[:, :-1], ((0, 0), (1, 0)))
    t_flat = np.ascontiguousarray(t, dtype=np.int32).reshape(-1)
    p_flat = np.ascontiguousarray(prev, dtype=np.int32).reshape(-1)
    idx = _hash_idx_host(t_flat, p_flat)
    row_counts = np.bincount(idx, minlength=SZ)
    rows_per_core = _balance_rows(row_counts)
    owner_of_row = np.empty(SZ, np.int64)
    loc_of_row = np.empty(SZ, np.int64)
    for c, rows in enumerate(rows_per_core):
        owner_of_row[rows] = c
        loc_of_row[rows] = np.arange(SLICE)
    owner = owner_of_row[idx]
    loc = loc_of_row[idx]
    order = np.argsort(owner * SLICE + loc, kind="stable")
    counts = np.bincount(owner, minlength=N_CORES)
    tiles = max(1, int(-(-counts.max() // 128)))
    return idx, loc, owner, order, counts, tiles, rows_per_core


def make_in_maps(t, tab, w_proj, nq=2):
    """Host-side marshalling: route tokens, shard table rows, transpose."""
    tab = np.ascontiguousarray(np.asarray(tab), dtype=np.float32)
    w_proj = np.ascontiguousarray(np.asarray(w_proj), dtype=np.float32)
    idx, loc, owner, order, counts, tiles, rows_per_core = route(t)
    cap = tiles * 128

    import ml_dtypes
    bf16 = ml_dtypes.bfloat16
    tabT = np.ascontiguousarray(tab.T)                       # [D, SZ]
    # [k', q*KC*qw + kc*qw + d'] = w_proj.T[kc*128 + k', q*qw + d']
    qw = D // nq
    wTh = np.ascontiguousarray(
        np.ascontiguousarray(w_proj.T)
        .reshape(KC, 128, nq, qw).transpose(1, 2, 0, 3).reshape(128, nq * KC * qw)
    ).astype(bf16)
    iota3 = (np.arange(128, dtype=np.float32)[:, None]
             + 128.0 * np.arange(RC_LOC, dtype=np.float32)[None, :])

    in_maps = []
    ranges_per_core = []
    off = 0
    for c in range(N_CORES):
        n = int(counts[c])
        toks = order[off: off + n]
        off += n
        loc_sh = np.full(cap, SLICE - 1, np.int64)
        loc_sh[:n] = loc[toks]
        rng = tuple(
            (int(loc_sh[j * 128:(j + 1) * 128].min() // 128),
             int(loc_sh[j * 128:(j + 1) * 128].max() // 128))
            for j in range(tiles)
        )
        ranges_per_core.append(rng)
        idxf = loc_sh.astype(np.float16)[None, :]
        # [rc][k'][kc*128 + r] = tab[rows_c[rc*128 + r], kc*128 + k']
        tabT_sl = np.ascontiguousarray(
            tabT[:, rows_per_core[c]]
            .reshape(KC, 128, RC_LOC, 128)
            .transpose(2, 1, 0, 3)
            .reshape(RC_LOC, 128, KC * 128)
        ).astype(bf16)
        in_maps.append(
            {"idxf": idxf, "iota": iota3, "tabT": tabT_sl, "wTh": wTh}
        )
    # SPMD: one program for all cores — union the chunk ranges over cores
    bmax = tuple(
        (min(ranges_per_core[c][j][0] for c in range(N_CORES)),
         max(ranges_per_core[c][j][1] for c in range(N_CORES)))
        for j in range(tiles)
    )
    return in_maps, order, counts, tiles, bmax


def kernel(t, tab, w_proj):
    in_maps, order, counts, tiles, bmax = make_in_maps(t, tab, w_proj)
    nc = build(tiles, bmax=bmax)
    res = run_bass_kernel_spmd(nc, in_maps, list(range(N_CORES)))
    out = np.empty((NTOK, D), np.float32)
    off = 0
    for c in range(N_CORES):
        n = int(counts[c])
        # out_sh [tiles, 128, D] is already token-major
        o = np.asarray(res.results[c]["out_sh"]).reshape(-1, D)
        out[order[off: off + n]] = o[:n]
        off += n
    return out.reshape(B, T, D)


# revision 32
# speedup vs baseline: 1.2042x; 1.2042x over previous
"""Trainium2 Bass kernel for nn_BigramHash: out = tab[hash(t,prev)] @ w_proj.T.

Strategy (fold sharded by table rows, tokens routed to row owners, gather
done as a one-hot matmul from SBUF-resident tab2 — no HBM gather traffic):

  - The projection is folded into the table on-device with bf16 matmuls:
    tab2 = tab @ w_proj.T, sharded by table rows.  Rows are assigned to
    cores by a balanced partition (exactly 384 rows per core, token
    counts equalized to NTOK/8) so every core gets exactly `tiles`
    128-token tiles of output — no padding waste.
  - The host routes each token to the core owning its hashed row and
    ships the LOCAL row index per token as flat fp16 (the hash is pure
    routing/marshalling — the host already computes it for the argsort).
  - The fold and the gather both run in nq=2 column halves with w_proj
    loaded half-major: the first fold needs only half the weights, so
    the first output DMA starts after ~1.25 MB of input instead of
    2.25 MB.  Critical loads (tabT chunk 0 + w half 0) go on the SP
    HWDGE queue (dodging the ACT LoadActFuncSet); everything else loads
    on ACT, overlapping the output stream.
  - Per 128-token tile and half: a K=1 fp16 matmul broadcasts the
    tile's indices across partitions (PSUM), is_equal against a
    per-partition iota builds a one-hot matrix (bf16, kept in SBUF for
    both halves), and one-hot @ tab2_chunk matmuls materialize the
    gathered rows in PSUM; DVE/ACT copy them to SBUF and the SP queue
    streams them to HBM in 512 KB chunks (2 tiles x 1 half).  The
    output stream is the roofline term (~16.8 MB/core); everything
    else hides behind it.

HBM traffic per core: ~2.8 MB of weights/indices in + ~16.8 MB out.
"""

import numpy as np

import concourse.bass as bass
import concourse.tile as tile
from concourse import bacc, mybir
from concourse.bass_utils import run_bass_kernel_spmd

N_CORES = 8
B, T = 4, 8192
SZ, D = 3072, 1024
NTOK = B * T                      # 32768
SLICE = SZ // N_CORES             # 384 table rows per core
RC_LOC = SLICE // 128             # 3 row chunks per core
KC = D // 128                     # 8 contraction chunks
HW = D // 2                       # 512-column half

C_T = 31337 % SZ                  # 617
C_P = 1000003 % SZ                # 1603

_CACHE = {}


def declare_io(nc, tiles, nq=2):
    f32 = mybir.dt.float32
    f16 = mybir.dt.float16
    bf16 = mybir.dt.bfloat16
    cap = tiles * 128
    qw = D // nq
    # flat local row index per token, fp16 (exact: values <= 383)
    idxf_ap = nc.dram_tensor("idxf", [1, cap], f16, kind="ExternalInput").ap()
    # per-partition iota p + 128c for the one-hot compare
    iota_ap = nc.dram_tensor("iota", [128, RC_LOC], f32, kind="ExternalInput").ap()
    # tabT re-laid by row-chunk: [rc][k', kc*128 + r]
    tabT_ap = nc.dram_tensor(
        "tabT", [RC_LOC, 128, KC * 128], bf16, kind="ExternalInput"
    ).ap()
    # w_proj.T column-slice-major: [k', q*KC*qw + kc*qw + d']
    wTh_ap = nc.dram_tensor("wTh", [128, nq * KC * qw], bf16,
                            kind="ExternalInput").ap()
    # output, tile/column-slice-major: host untangles to token order
    # output in bf16: every output element equals a bf16 tab2 entry
    # (one-hot gather), so the bf16 store + host f32 upcast is exact —
    # and it halves the dominant HBM stream
    out_ap = nc.dram_tensor(
        "out_sh", [tiles, 128, D], bf16, kind="ExternalOutput"
    ).ap()
    return idxf_ap, iota_ap, tabT_ap, wTh_ap, out_ap


def emit_body(nc, tc, io, tiles, bmax=None, gather_bufs=6, oc=2,
              lookahead=2, fold_spread=3, wh_pieces=1, dve_head=0,
              out_rings=1, copy_split=1, opool_bufs=2, bpool_bufs=2,
              fpool_bufs=2, nq=2, first_small=0, tt0_act=1,
              fw_from=18, fc=1, split_first=0):
    f32 = mybir.dt.float32
    f16 = mybir.dt.float16
    bf16 = mybir.dt.bfloat16
    idxf_ap, iota_ap, tabT_ap, wTh_ap, out_ap = io
    cap = tiles * 128
    qw = D // nq
    if bmax is None:
        ranges = [(0, RC_LOC - 1)] * tiles
    else:
        ranges = list(bmax)
    GR = 4                        # tiles per index-broadcast group
    n_groups = -(-tiles // GR)
    grange = [
        (min(ranges[j][0] for j in range(g * GR, min((g + 1) * GR, tiles))),
         max(ranges[j][1] for j in range(g * GR, min((g + 1) * GR, tiles))))
        for g in range(n_groups)
    ]
    n_oh = sum(hi - lo + 1 for lo, hi in grange)
    with (
        tc.tile_pool(name="weights", bufs=1) as wpool,
        tc.tile_pool(name="idx", bufs=1) as ipool,
        tc.tile_pool(name="oh", bufs=n_oh) as ohpool,
        tc.tile_pool(name="gather", bufs=gather_bufs) as gpool,
        tc.tile_pool(name="bcast_ps", bufs=bpool_bufs, space="PSUM") as bpool,
        tc.tile_pool(name="fold_ps", bufs=fpool_bufs, space="PSUM") as fpool,
        tc.tile_pool(name="gather_ps", bufs=opool_bufs, space="PSUM") as opool,
    ):
        # ---- loads. SP queue: the h0-critical weights, then the output
        # stream.  ACT queue: LoadActFuncSet (framework preamble), the
        # small index/iota tensors, tabT rc1/rc2, then the h1 weights —
        # those overlap the h0 output stream on a separate HWDGE ring.
        tabT_sb = [
            wpool.tile([128, KC * 128], bf16, tag=f"tabT{rc}", name=f"tT{rc}")
            for rc in range(RC_LOC)
        ]
        wTh_sb = wpool.tile([128, nq * KC * qw], bf16, tag="wTh", name="wTh")
        if not tt0_act:
            nc.sync.dma_start(tabT_sb[0][:], tabT_ap[0])
        hstep = KC * qw // wh_pieces
        for pc in range(wh_pieces):
            nc.sync.dma_start(wTh_sb[:, pc * hstep:(pc + 1) * hstep],
                              wTh_ap[:, pc * hstep:(pc + 1) * hstep])
        if tt0_act:
            nc.scalar.dma_start(tabT_sb[0][:], tabT_ap[0])
        idx_sb = ipool.tile([1, cap], f16, name="idxf")
        nc.scalar.dma_start(idx_sb[:], idxf_ap[:])
        iota_sb = ipool.tile([128, RC_LOC], f32, name="iota")
        nc.scalar.dma_start(iota_sb[:], iota_ap[:])
        nc.scalar.dma_start(tabT_sb[1][:], tabT_ap[1])
        nc.scalar.dma_start(tabT_sb[2][:], tabT_ap[2])

        # weight loads for column slices q >= 1 are deferred into the
        # gather pass so the early pair copies (also on ACT) are not stuck
        # behind them in the in-order ACT queue; they still land long
        # before their pass needs them.
        def _wh1_piece(qpc):
            qq, pc = qpc
            o = qq * KC * qw
            nc.scalar.dma_start(wTh_sb[:, o + pc * hstep:o + (pc + 1) * hstep],
                                wTh_ap[:, o + pc * hstep:o + (pc + 1) * hstep])
        wh1_pending = [(qq, pc) for qq in range(1, nq)
                       for pc in range(wh_pieces)]

        ones_sb = wpool.tile([1, 128], f16)
        nc.vector.memset(ones_sb[:], 1.0)

        # PE p-state warmup: early dummy matmuls start the ramp clock so
        # the fold runs at full clock when its inputs land
        for _ in range(3):
            psw = bpool.tile([128, GR * 128], f32, name="psb", tag="psb")
            nc.tensor.matmul(psw[:, 0:128], ones_sb[0:1, 0:128], ones_sb[:],
                             start=True, stop=True)

        # ---- fold: tab2[rc][:, h*512:...] = tab[rows rc] @ w_proj.T[h]
        # h-major queue: all of h0 (rc0, rc1, rc2) before h1, emitted
        # incrementally so the PE pipeline never pauses for a burst.
        tab2_sb = [
            wpool.tile([128, D], bf16, tag=f"tab2_{rc}", name=f"tab2_{rc}")
            for rc in range(RC_LOC)
        ]
        fold_queue = [(h, rc, kc)
                      for h in range(nq) for rc in range(RC_LOC)
                      for kc in range(KC)]
        fold_state = {"pos": 0, "ps": None, "ncopies": 0}

        def emit_fold(n=None, upto=None):
            while fold_state["pos"] < len(fold_queue):
                h, rc, kc = fold_queue[fold_state["pos"]]
                if upto is not None and (h, rc) > upto:
                    break
                if n is not None:
                    if n <= 0:
                        break
                    n -= 1
                if kc == 0:
                    fold_state["ps"] = fpool.tile([128, qw], f32, name="fps",
                                                  tag="fps")
                nc.tensor.matmul(
                    fold_state["ps"][:],
                    tabT_sb[rc][:, kc * 128:(kc + 1) * 128],
                    wTh_sb[:, h * KC * qw + kc * qw:h * KC * qw + (kc + 1) * qw],
                    start=(kc == 0), stop=(kc == KC - 1),
                )
                if kc == KC - 1:
                    # first copy (h0 rc0) on DVE — ACT is still loading;
                    # the rest on ACT once its queue drains
                    eng = nc.vector if fold_state["ncopies"] == 0 else nc.scalar
                    if eng is nc.scalar:
                        eng.copy(tab2_sb[rc][:, h * qw:(h + 1) * qw],
                                 fold_state["ps"][:])
                    else:
                        eng.tensor_copy(tab2_sb[rc][:, h * qw:(h + 1) * qw],
                                        fold_state["ps"][:])
                    fold_state["ncopies"] += 1
                fold_state["pos"] += 1

        # ---- one-hot selection matrices, kept in SBUF for both halves ----
        ohs = {}

        def emit_bcast_eq(g):
            gn = min(GR, tiles - g * GR)
            psb = bpool.tile([128, GR * 128], f32, name="psb", tag="psb")
            nc.tensor.matmul(
                psb[:, 0:gn * 128], ones_sb[0:1, 0:128],
                idx_sb[0:1, g * GR * 128:(g * GR + gn) * 128],
                start=True, stop=True,
            )
            glo, ghi = grange[g]
            ohs[g] = {}
            for c in range(glo, ghi + 1):
                oh = ohpool.tile([128, GR * 128], bf16, name="oh", tag="oh")
                nc.vector.tensor_tensor(
                    oh[:, 0:gn * 128], psb[:, 0:gn * 128],
                    iota_sb[:, c:c + 1].to_broadcast([128, gn * 128]),
                    op=mybir.AluOpType.is_equal,
                )
                ohs[g][c] = oh

        emit_fold(upto=(0, 0))

        # ---- gather schedule ----
        # Default: all tiles at h=0, then h=1 (half-width, two phases).
        # With fw_from=S (nq=2): tiles 0..S-1 run half-width in two
        # phases (early stream start while the h1 weights load), tiles
        # S.. run full-width in a single pass — full-tile output DMAs
        # write 4 KB per-partition runs (half the descriptor count).
        def chunk_sched(h, jlo, jhi, step):
            sched, j0 = [], jlo
            if h == 0:
                for _ in range(first_small):
                    if j0 < jhi:
                        sched.append((j0, 1))
                        j0 += 1
            while j0 < jhi:
                k = min(step, jhi - j0)
                sched.append((j0, k))
                j0 += k
            return sched

        copy_engs = [nc.vector, nc.scalar]
        lookahead = max(1, min(lookahead, n_groups))
        for g in range(lookahead):
            emit_bcast_eq(g)
        state = {"unit": 0, "chunk": 0, "item": 0}

        def ensure_oh(gi):
            need = min(gi + lookahead, n_groups - 1)
            while len(ohs) <= need:
                emit_bcast_eq(len(ohs))

        def copy_eng():
            if state["unit"] < dve_head:
                eng = nc.vector
            else:
                eng = copy_engs[state["unit"] % 2]
            state["unit"] += 1
            if wh1_pending and state["unit"] >= 2:
                _wh1_piece(wh1_pending.pop(0))
            return eng

        def emit_copy(eng, dst, src):
            if eng is nc.scalar:
                eng.copy(dst, src)
            else:
                eng.tensor_copy(dst, src)

        def out_ring():
            ring_engs = [nc.sync, nc.scalar][:out_rings]
            eng = ring_engs[state["chunk"] % len(ring_engs)]
            state["chunk"] += 1
            return eng

        def spread(early_gate):
            if state["item"] >= 1:
                gate = ((0, RC_LOC - 1)
                        if (early_gate and state["item"] < 12) else None)
                emit_fold(n=fold_spread, upto=gate)
            state["item"] += 1

        def half_phase(h, jlo, jhi):
            sched = chunk_sched(h, jlo, jhi, oc)
            tile2chunk = {}
            for ci, (s, k) in enumerate(sched):
                for j in range(s, s + k):
                    tile2chunk[j] = ci
            g_for_chunk = {}
            ps = None
            pk = 1
            for j in range(jlo, jhi):
                gi, off = j // GR, j % GR
                cmin, cmax = ranges[j]
                emit_fold(upto=(h, cmax))
                jj = (j - jlo) % 2
                if jj == 0:
                    pk = min(2, jhi - j)
                    ps = opool.tile([128, 2 * qw], f32, name="ps")
                for c in range(cmin, cmax + 1):
                    nc.tensor.matmul(
                        ps[:, jj * qw:(jj + 1) * qw],
                        ohs[gi][c][:, off * 128:(off + 1) * 128],
                        tab2_sb[c][:, h * qw:(h + 1) * qw],
                        start=(c == cmin), stop=(c == cmax),
                    )
                spread(h == 0)
                ensure_oh(gi)
                # copy unit = contiguous tiles within one PSUM pair AND
                # one DMA chunk; fires at whichever boundary comes first
                ci = tile2chunk[j]
                s, k = sched[ci]
                pair_start = j - jj
                pair_end = min(pair_start + pk - 1, jhi - 1)
                chunk_end = s + k - 1
                if j == min(pair_end, chunk_end):
                    unit_start = max(pair_start, s)
                    if ci not in g_for_chunk:
                        g_for_chunk[ci] = gpool.tile([128, oc * qw], bf16,
                                                     name="g_sb")
                    g_sb = g_for_chunk[ci]
                    gb = (unit_start - s) * qw
                    pb = (unit_start - pair_start) * qw
                    width = j - unit_start + 1
                    if split_first and state["unit"] == 0 and width == 2:
                        # split the very first unit across both engines so
                        # the first out DMA waits a half-size copy
                        e1, e2 = copy_eng(), copy_eng()
                        emit_copy(e1, g_sb[:, gb:gb + qw],
                                  ps[:, pb:pb + qw])
                        emit_copy(e2, g_sb[:, gb + qw:gb + 2 * qw],
                                  ps[:, pb + qw:pb + 2 * qw])
                    else:
                        emit_copy(copy_eng(),
                                  g_sb[:, gb:(j - s + 1) * qw],
                                  ps[:, pb:(j - pair_start + 1) * qw])
                if j == chunk_end:
                    g_sb = g_for_chunk.pop(ci)
                    out_ring().dma_start(
                        out_ap[s:s + k, :, h * qw:(h + 1) * qw]
                        .rearrange("k p d -> p k d"),
                        g_sb[:, 0:k * qw].rearrange("p (k d) -> p k d",
                                                    k=k),
                    )

        def full_span(jlo):
            g_sb = None
            fk = 1
            for j in range(jlo, tiles):
                gi, off = j // GR, j % GR
                cmin, cmax = ranges[j]
                emit_fold(upto=(nq - 1, cmax))
                ps = opool.tile([128, 2 * qw], f32, name="ps")
                for hh in range(2):
                    for c in range(cmin, cmax + 1):
                        nc.tensor.matmul(
                            ps[:, hh * qw:(hh + 1) * qw],
                            ohs[gi][c][:, off * 128:(off + 1) * 128],
                            tab2_sb[c][:, hh * qw:(hh + 1) * qw],
                            start=(c == cmin), stop=(c == cmax),
                        )
                spread(False)
                ensure_oh(gi)
                qj = (j - jlo) % fc
                if qj == 0:
                    fk = min(fc, tiles - j)
                    g_sb = gpool.tile([128, fc * D], bf16, name="g_sb")
                emit_copy(copy_eng(), g_sb[:, qj * D:(qj + 1) * D], ps[:])
                if qj == fk - 1:
                    j0 = j - qj
                    out_ring().dma_start(
                        out_ap[j0:j0 + fk].rearrange("k p d -> p k d"),
                        g_sb[:, 0:fk * D].rearrange("p (k d) -> p k d",
                                                    k=fk),
                    )

        if fw_from is not None and nq == 2 and 0 < fw_from < tiles:
            S = min(fw_from + (fw_from % 2), tiles)
            half_phase(0, 0, S)
            half_phase(1, 0, S)
            full_span(S)
        else:
            for h in range(nq):
                half_phase(h, 0, tiles)
        emit_fold()


def build(tiles, loop_iters=None, bmax=None, unroll=1, **body_kw):
    """Build the SPMD Bass program (same program for all 8 cores)."""
    key = ("nc", tiles, loop_iters, bmax, unroll, tuple(sorted(body_kw.items())))
    if key in _CACHE:
        return _CACHE[key]
    nc = bacc.Bacc("TRN2", target_bir_lowering=False, debug=False)
    io = declare_io(nc, tiles, nq=body_kw.get("nq", 2))
    with tile.TileContext(nc) as tc:
        if loop_iters is None:
            emit_body(nc, tc, io, tiles, bmax=bmax, **body_kw)
        else:
            with tc.For_i(0, loop_iters, 1):
                for _ in range(unroll):
                    emit_body(nc, tc, io, tiles, bmax=bmax, **body_kw)
    nc.compile()
    _CACHE[key] = nc
    return nc


def _hash_idx_host(t_flat, p_flat):
    a = (t_flat.astype(np.int64) % SZ) * C_T
    b = (p_flat.astype(np.int64) % SZ) * C_P
    return ((a + b) % SZ).astype(np.int64)


def _balance_rows(row_counts):
    """Partition SZ rows into N_CORES sets of exactly SLICE rows with token
    counts as equal as possible (ideally == NTOK/N_CORES each)."""
    target = int(row_counts.sum()) // N_CORES
    order = np.argsort(-row_counts, kind="stable")
    lists = [[] for _ in range(N_CORES)]
    sums = [0] * N_CORES
    for r in order:
        best = min(
            (c for c in range(N_CORES) if len(lists[c]) < SLICE),
            key=lambda c: (sums[c], c),
        )
        lists[best].append(int(r))
        sums[best] += int(row_counts[r])
    # pairwise swap repair toward max(sums) <= target
    for _ in range(400):
        hi = int(np.argmax(sums))
        if sums[hi] <= target:
            break
        lo = int(np.argmin(sums))
        want = min(sums[hi] - target, target - sums[lo])
        if want <= 0:
            break
        lo_vals = {}
        for bi, b in enumerate(lists[lo]):
            lo_vals.setdefault(int(row_counts[b]), bi)
        done = False
        for d in range(int(want), 0, -1):
            for ai, a in enumerate(lists[hi]):
                bi = lo_vals.get(int(row_counts[a]) - d)
                if bi is not None:
                    b = lists[lo][bi]
                    lists[hi][ai], lists[lo][bi] = b, a
                    sums[hi] -= d
                    sums[lo] += d
                    done = True
                    break
            if done:
                break
        if not done:
            break
    return [np.sort(np.array(l, dtype=np.int64)) for l in lists]


def route(t, tab=None, w_proj=None):
    """Host routing: balanced row->core map, tokens ordered by owning core
    then local row; returns order, per-core counts, capacity in tiles."""
    t = np.asarray(t)
    prev = np.pad(t[:, :-1], ((0, 0), (1, 0)))
    t_flat = np.ascontiguousarray(t, dtype=np.int32).reshape(-1)
    p_flat = np.ascontiguousarray(prev, dtype=np.int32).reshape(-1)
    idx = _hash_idx_host(t_flat, p_flat)
    row_counts = np.bincount(idx, minlength=SZ)
    rows_per_core = _balance_rows(row_counts)
    owner_of_row = np.empty(SZ, np.int64)
    loc_of_row = np.empty(SZ, np.int64)
    for c, rows in enumerate(rows_per_core):
        owner_of_row[rows] = c
        loc_of_row[rows] = np.arange(SLICE)
    owner = owner_of_row[idx]
    loc = loc_of_row[idx]
    order = np.argsort(owner * SLICE + loc, kind="stable")
    counts = np.bincount(owner, minlength=N_CORES)
    tiles = max(1, int(-(-counts.max() // 128)))
    return idx, loc, owner, order, counts, tiles, rows_per_core


def make_in_maps(t, tab, w_proj, nq=2):
    """Host-side marshalling: route tokens, shard table rows, transpose."""
    tab = np.ascontiguousarray(np.asarray(tab), dtype=np.float32)
    w_proj = np.ascontiguousarray(np.asarray(w_proj), dtype=np.float32)
    idx, loc, owner, order, counts, tiles, rows_per_core = route(t)
    cap = tiles * 128

    import ml_dtypes
    bf16 = ml_dtypes.bfloat16
    tabT = np.ascontiguousarray(tab.T)                       # [D, SZ]
    # [k', q*KC*qw + kc*qw + d'] = w_proj.T[kc*128 + k', q*qw + d']
    qw = D // nq
    wTh = np.ascontiguousarray(
        np.ascontiguousarray(w_proj.T)
        .reshape(KC, 128, nq, qw).transpose(1, 2, 0, 3).reshape(128, nq * KC * qw)
    ).astype(bf16)
    iota3 = (np.arange(128, dtype=np.float32)[:, None]
             + 128.0 * np.arange(RC_LOC, dtype=np.float32)[None, :])

    in_maps = []
    ranges_per_core = []
    off = 0
    for c in range(N_CORES):
        n = int(counts[c])
        toks = order[off: off + n]
        off += n
        loc_sh = np.full(cap, SLICE - 1, np.int64)
        loc_sh[:n] = loc[toks]
        rng = tuple(
            (int(loc_sh[j * 128:(j + 1) * 128].min() // 128),
             int(loc_sh[j * 128:(j + 1) * 128].max() // 128))
            for j in range(tiles)
        )
        ranges_per_core.append(rng)
        idxf = loc_sh.astype(np.float16)[None, :]
        # [rc][k'][kc*128 + r] = tab[rows_c[rc*128 + r], kc*128 + k']
        tabT_sl = np.ascontiguousarray(
            tabT[:, rows_per_core[c]]
            .reshape(KC, 128, RC_LOC, 128)
            .transpose(2, 1, 0, 3)
            .reshape(RC_LOC, 128, KC * 128)
        ).astype(bf16)
        in_maps.append(
            {"idxf": idxf, "iota": iota3, "tabT": tabT_sl, "wTh": wTh}
        )
    # SPMD: one program for all cores — union the chunk ranges over cores
    bmax = tuple(
        (min(ranges_per_core[c][j][0] for c in range(N_CORES)),
         max(ranges_per_core[c][j][1] for c in range(N_CORES)))
        for j in range(tiles)
    )
    return in_maps, order, counts, tiles, bmax


def kernel(t, tab, w_proj):
    in_maps, order, counts, tiles, bmax = make_in_maps(t, tab, w_proj)
    nc = build(tiles, bmax=bmax)
    res = run_bass_kernel_spmd(nc, in_maps, list(range(N_CORES)))
    out = np.empty((NTOK, D), np.float32)
    off = 0
    for c in range(N_CORES):
        n = int(counts[c])
        # out_sh [tiles, 128, D] is already token-major; bf16 -> f32
        # upcast is exact (the device values are bf16 tab2 entries)
        o = np.asarray(res.results[c]["out_sh"]).reshape(-1, D)
        o = o.astype(np.float32)
        out[order[off: off + n]] = o[:n]
        off += n
    return out.reshape(B, T, D)
